# revision 65
# baseline (speedup 1.0000x reference)
"""Multi-head attention (B=2, S=2048, D=1024, H=16) on 8 trn2 NeuronCores.

Tensor-parallel over heads (2 heads per core, column-sliced wq/wk/wv) for the
QKV projections and attention; a per-(batch, s-half) AllToAll redistributes
the attention output so each core computes the output projection for its own
interleaved 128-col chunks of the flattened (B*S) sequence.

Schedule (single PE stream, ACT does exp only, DVE does all copies/normalize):
  - x streams in nb-major 1MB chunks so the first matmul starts ~4us in;
    K projects first so logits/exp can start early
  - V is computed directly in natural [t, dl] layout (stationary x-chunks,
    moving wv) so no PE transposes are needed; bv folds into bo on the host
  - logits are computed transposed [t, s]; exp (ACT) feeds the P@V matmul;
    ones-columns interleaved with V produce softmax denominators in the same
    PV matmuls; normalize reads PV psum directly (reciprocal+mul on DVE)
  - batch-1 QKV interleaves into batch-0's ACT-paced attention window as PE
    filler; batch-0's output projection fills batch-1's second-half window
  - AllToAlls fire per (batch, s-half) as soon as both heads normalize; only
    the last collective plus one 128-col projection chunk remain in the tail
"""

import sys

sys.path.insert(0, "/opt/trn_rl_repo")

import numpy as np

import concourse.mybir as mybir
import concourse.tile as tile
from concourse import bacc
from concourse.bass_utils import run_bass_kernel_spmd

B, S, D = 2, 2048, 1024
H, HD = 16, 64
NCORES = 8
DL = D // NCORES          # 128 local attn dims (2 heads) per core
R = B * S                 # 4096 flattened rows
RSL = R // NCORES         # 512 output rows per core
P = 128
KC = D // P               # 8 contraction chunks of 128
TC = S // P               # 16 key/t chunks per batch
SB = 512                  # moving-operand (N) tile for QKV / logits
NB = S // SB              # 4 nb chunks per batch half of x
SH = S // 2               # 1024-wide s half
CW = 128                  # per-core output column chunk (per batch, sh)
F32 = mybir.dt.float32
F32R = mybir.dt.float32r
F16 = mybir.dt.float16

_CACHE = {}


def _build(n_iters=1, phases=3, bench=False):
    nc = bacc.Bacc("TRN2", target_bir_lowering=False, debug=False,
                   num_devices=NCORES)
    Exp = mybir.ActivationFunctionType.Exp

    kind = "Internal" if bench else "ExternalInput"
    xT = nc.dram_tensor("xT", [D, R], F16, kind=kind)
    wqT = nc.dram_tensor("wqT", [P, D], F16, kind=kind)
    wkT = nc.dram_tensor("wkT", [P, D], F16, kind=kind)
    wvT = nc.dram_tensor("wvT", [P, D], F16, kind=kind)
    woT = nc.dram_tensor("woT", [P, KC * D], F16, kind=kind)
    bqk = nc.dram_tensor("bqk", [DL, 2], F32, kind=kind)
    bo_t = nc.dram_tensor("bo_t", [P, KC], F32, kind=kind)
    out = nc.dram_tensor("out", [D, RSL], F32, kind="ExternalOutput")

    with tile.TileContext(nc) as tc:
        with (
            tc.tile_pool(name="const", bufs=1) as const,
            tc.tile_pool(name="persist", bufs=1) as persist,
            tc.tile_pool(name="dram", bufs=1, space="DRAM") as dram,
        ):
            bias2 = const.tile([DL, 2], F32, tag="bias2")
            bo_s = const.tile([P, KC], F32, tag="bo_s")
            w_s = {}
            for name in ("wq", "wk", "wv"):
                w_s[name] = const.tile([P, D], F16, tag=f"w_{name}",
                                       name=f"w_{name}")
            wo_all = const.tile([P, KC * D], F16, tag="wo_all")
            wo_s = [wo_all[:, kc * D:(kc + 1) * D] for kc in range(KC)]

            def load_w(t, wt):
                if bench:
                    nc.vector.memset(t[:], 0.0)
                else:
                    nc.sync.dma_start(t[:], wt[:, :])

            # wk first: the K projection runs first so exp starts early
            load_w(w_s["wk"], wkT)

            # persistent activations
            QT = persist.tile([P, R], F16, tag="QT")   # [2 heads*64, B*S]
            KT = persist.tile([P, R], F16, tag="KT")
            # V natural per 128-row t-chunk: [v_h0 |ones| v_h1 |ones]
            vn = persist.tile([P, (R // P) * 256], F16, tag="vn")
            vn3 = vn[:].rearrange("p (g two c) -> p g two c", two=2, c=128)
            nc.vector.memset(vn3[:, :, :, 64:128], 1.0)
            attnTs = [persist.tile([P, R], F16, tag=f"attnT{h}",
                                   name=f"attnT{h}") for h in range(2)]

            for it in range(n_iters):
                a2a_in = [[dram.tile([NCORES, P, CW], F16,
                                     tag=f"a2a_in{it}_{b}_{sh}",
                                     name=f"a2a_in{it}_{b}_{sh}")
                           for sh in range(2)] for b in range(B)]
                a2a_out = [[dram.tile([NCORES, P, CW], F16,
                                      tag=f"a2a_out{it}_{b}_{sh}",
                                      name=f"a2a_out{it}_{b}_{sh}")
                            for sh in range(2)] for b in range(B)]

                with tc.tile_pool(name=f"xt{it}", bufs=2) as xt_pool:
                    xth = [xt_pool.tile([P, KC * S], F16, tag="xt",
                                        name=f"xt_{it}_{half}")
                           for half in range(2)]

                    def xs(half, kc):
                        return xth[half][:, kc * S:(kc + 1) * S]

                    def load_x(half):
                        # nb-major so the first 1MB arrives fast; one
                        # strided DMA per nb covers all kc chunks
                        hof = half * S
                        xv = xth[half][:].rearrange("p (kc s) -> p kc s", s=S)
                        srcv = xT.rearrange("(kc p) r -> p kc r", p=P)
                        for nb in range(NB):
                            nsplit = 4 if (half == 0 and nb == 0) else 2
                            for kh in range(nsplit):
                                kq = KC // nsplit
                                ks = slice(kh * kq, (kh + 1) * kq)
                                nc.sync.dma_start(
                                    xv[:, ks, nb * SB:(nb + 1) * SB],
                                    srcv[:, ks,
                                         hof + nb * SB:hof + (nb + 1) * SB])
                            if half == 0 and nb == 0:
                                load_w(w_s["wq"], wqT)
                                load_w(w_s["wv"], wvT)
                                if bench:
                                    nc.vector.memset(bias2[:], 0.0)
                                    nc.vector.memset(bo_s[:], 0.0)
                                else:
                                    nc.sync.dma_start(bias2[:], bqk[:])
                                    nc.sync.dma_start(bo_s[:], bo_t[:])

                    def qk_group(pj, half, nb, pool, tag):
                        # one [128dl, 512rows] psum group + DVE copy w/ bias
                        name, dst = (("wk", KT), ("wq", QT))[pj]
                        ps = pool.tile([P, SB], F32, tag=tag,
                                       name=f"qk_{it}_{half}_{pj}_{nb}")
                        w = w_s[name]
                        for kc in range(KC):
                            nc.tensor.matmul(
                                ps[:], w[:, kc * P:(kc + 1) * P],
                                xs(half, kc)[:, nb * SB:(nb + 1) * SB],
                                start=(kc == 0), stop=(kc == KC - 1))
                        o = half * S + nb * SB
                        nc.vector.tensor_scalar_add(
                            dst[:, o:o + SB], ps[:],
                            bias2[:, 1 - pj:2 - pj])

                    def v_group(half, q, pool, tag):
                        # natural-V for 4 t-chunks: psum [128t, 4*128dl]
                        ps = pool.tile([P, SB], F32, tag=tag,
                                       name=f"vq_{it}_{half}_{q}")
                        wv = w_s["wv"]
                        for i in range(4):
                            tcn = q * 4 + i
                            for kc in range(KC):
                                nc.tensor.matmul(
                                    ps[:, i * P:(i + 1) * P],
                                    xs(half, kc)[:, tcn * P:(tcn + 1) * P],
                                    wv[:, kc * P:(kc + 1) * P],
                                    start=(kc == 0), stop=(kc == KC - 1))
                        for i in range(4):
                            g = half * TC + q * 4 + i
                            o = g * 256
                            nc.vector.tensor_copy(
                                vn[:, o:o + 64], ps[:, i * P:i * P + 64])
                            nc.vector.tensor_copy(
                                vn[:, o + 128:o + 192],
                                ps[:, i * P + 64:(i + 1) * P])

                    norm_muls = {}

                    def attention_batch(b, ps3, exps, norm, aux=None):
                        base = b * S
                        for sh in range(2):
                            for h in range(2):
                                if sh == 0 and h == 1 and aux is not None:
                                    with tc.high_priority(offset=400):
                                        qk_group(1, b, 2, aux, "aux")
                                        qk_group(1, b, 3, aux, "aux")
                                hr = slice(h * HD, (h + 1) * HD)
                                sof = base + sh * SH
                                pv = ps3.tile([P, SH], F32, tag="pv", bufs=1,
                                              name=f"pv_{it}_{b}_{h}_{sh}")
                                for tcn in range(TC):
                                    ex = exps.tile(
                                        [P, SH], F16, tag="ex",
                                        name=f"ex_{it}_{b}_{h}_{sh}_{tcn}")
                                    lg = ps3.tile(
                                        [P, SH], F32, tag="lg", bufs=2,
                                        name=f"lg_{it}_{b}_{h}_{sh}_{tcn}")
                                    with tc.high_priority(offset=400):
                                        for sb in range(2):
                                            nc.tensor.matmul(
                                                lg[:, sb * SB:(sb + 1) * SB],
                                                KT[hr, base + tcn * P:
                                                   base + (tcn + 1) * P],
                                                QT[hr, sof + sb * SB:
                                                   sof + (sb + 1) * SB],
                                                start=True, stop=True)
                                        if (b == 1 and sh == 1
                                                and h == 1
                                                and tcn == TC - 1):
                                            for se in range(2):
                                                el = slice(se * SB,
                                                           (se + 1) * SB)
                                                nc.scalar.activation(
                                                    ex[:, el], lg[:, el],
                                                    Exp, scale=1.0 / 8.0)
                                        else:
                                            nc.scalar.activation(
                                                ex[:], lg[:], Exp,
                                                scale=1.0 / 8.0)
                                        o = (b * TC + tcn) * 256 + h * 128
                                        for sb in range(2):
                                            nc.tensor.matmul(
                                                pv[:, sb * SB:(sb + 1) * SB],
                                                vn[:, o:o + 128],
                                                ex[:, sb * SB:
                                                   (sb + 1) * SB],
                                                start=(tcn == 0),
                                                stop=(tcn == TC - 1))
                                # normalize straight out of PV psum,
                                # per sb half so it pipelines with pv mms
                                rc = norm.tile([HD, SH], F32, tag="rc")
                                nq, qw = 2, SB
                                with tc.high_priority(offset=400):
                                    for sb in range(nq):
                                        sl = slice(sb * qw, (sb + 1) * qw)
                                        nc.vector.reciprocal(
                                            rc[:, sl], pv[64:128, sl])
                                        norm_muls[(b, sh, h, sb)] = \
                                            nc.vector.tensor_mul(
                                                attnTs[h][0:HD,
                                                          sof + sb * qw:
                                                          sof + (sb + 1) * qw],
                                                pv[0:64, sl], rc[:, sl])
                            # ship this (batch, s-half); overlaps compute
                            if phases >= 3:
                                stk = tc.high_priority(offset=400)
                                stk.__enter__()
                                for h2 in (1, 0):
                                    hs = slice(h2 * HD, (h2 + 1) * HD)
                                    nc.sync.dma_start(
                                        a2a_in[b][sh].rearrange(
                                            "j p c -> p j c")[hs],
                                        attnTs[h2][0:HD,
                                                   base + sh * SH:
                                                   base + (sh + 1) * SH]
                                        .rearrange("p (j c) -> p j c", c=CW))
                                nc.gpsimd.collective_compute(
                                    "AllToAll", mybir.AluOpType.bypass,
                                    replica_groups=[list(range(NCORES))],
                                    ins=[a2a_in[b][sh].opt()],
                                    outs=[a2a_out[b][sh].opt()])
                                stk.__exit__(None, None, None)

                    outv = out.rearrange("(mc p) c -> p mc c", p=P)

                    def proj_pass(b, sh, rh, ncols, coff, ps4, outs):
                        # 8 mc-chunks + one batched out DMA
                        ot = outs.tile([P, KC * ncols], F32, tag="ot",
                                       name=f"ot_{it}_{b}_{sh}")
                        for mc in range(KC):
                            ps = ps4.tile([P, SB], F32, tag="aux",
                                          name=f"ps4_{it}_{b}_{sh}_{mc}")
                            for kc in range(KC):
                                nc.tensor.matmul(
                                    ps[:, 0:ncols],
                                    wo_s[kc][:, mc * P:(mc + 1) * P],
                                    rh[:, kc * ncols:(kc + 1) * ncols],
                                    start=(kc == 0), stop=(kc == KC - 1))
                            nc.vector.tensor_scalar_add(
                                ot[:, mc * ncols:(mc + 1) * ncols],
                                ps[:, 0:ncols], bo_s[:, mc:mc + 1])
                        for mh in range(4):
                            ms = slice(mh * (KC // 4), (mh + 1) * (KC // 4))
                            nc.sync.dma_start(
                                outv[:, ms, coff:coff + ncols],
                                ot[:].rearrange("p (mc c) -> p mc c",
                                                c=ncols)[:, ms])

                    # ---- head: QKV-b0, K first, V natural; the attn
                    # psum pools coexist so the first logits/exp can be
                    # hoisted under the head by the scheduler ----
                    with (
                        tc.tile_pool(name=f"ps3{it}", bufs=1,
                                     space="PSUM") as ps3,
                        tc.tile_pool(name=f"exps{it}", bufs=16) as exps,
                        tc.tile_pool(name=f"norm{it}", bufs=2) as norm,
                        tc.tile_pool(name=f"aux{it}", bufs=2,
                                     space="PSUM") as aux,
                    ):
                        load_x(0)
                        qk_group(0, 0, 0, aux, "aux")  # K nb0
                        qk_group(1, 0, 0, aux, "aux")  # Q nb0
                        qk_group(1, 0, 1, aux, "aux")  # Q nb1
                        qk_group(0, 0, 1, aux, "aux")  # K nb1
                        qk_group(0, 0, 2, aux, "aux")  # K nb2
                        qk_group(0, 0, 3, aux, "aux")  # K nb3
                        for q in range(4):
                            v_group(0, q, aux, "aux")
                        load_x(1)

                        if bench:
                            nc.vector.memset(wo_all[:], 0.0)
                        else:
                            nc.sync.dma_start(wo_all[:], woT[:, :])
                        if phases < 2:
                            continue

                        attention_batch(0, ps3, exps, norm, aux)

                        # fillers drain into attention's PE slack:
                        # K/Q-s0 of b1 gate attn-b1's logits; V-b1 and
                        # Q-s1-b1 defer into the b1 window
                        for pj, nb in ((0, 0), (1, 0), (1, 1), (0, 1),
                                       (0, 2), (0, 3)):
                            qk_group(pj, 1, nb, aux, "aux")
                        for q in range(4):
                            v_group(1, q, aux, "aux")

                        if phases < 3:
                            continue

                        attention_batch(1, ps3, exps, norm, aux)

                        # output projection, one pass per (batch, s-half).
                        # rh DMAs gated (dep=) so the scheduler cannot hoist
                        # dependent matmuls before the collective really
                        # lands; ungated passes hoist into attn-b1 PE gaps.
                        with (
                            tc.tile_pool(name=f"proj{it}", bufs=1) as proj,
                            tc.tile_pool(name=f"outs{it}", bufs=2) as outs,
                        ):
                            # all but the last pass run under the final
                            # collective: gate their rh loads on the last
                            # norm so the scheduler cannot pull the matmuls
                            # into the (already saturated) attention windows
                            import bass_rust as _br
                            last_norm = norm_muls[(1, 1, 1, 1)]
                            for b in range(B):
                                for sh in range(2):
                                    rh = proj.tile([P, KC * CW], F16,
                                                   tag=f"rh{it}_{b}_{sh}",
                                                   name=f"rh{it}_{b}_{sh}")
                                    for kh in range(2):
                                        ks = slice(kh * (KC // 2),
                                                   (kh + 1) * (KC // 2))
                                        dma = nc.sync.dma_start(
                                            rh[:].rearrange(
                                                "p (kc c) -> p kc c",
                                                c=CW)[:, ks],
                                            a2a_out[b][sh].rearrange(
                                                "kc p c -> p kc c")[:, ks])
                                        if (b, sh) != (1, 1):
                                            dma.ins.add_dependency(
                                                last_norm.ins.name,
                                                _br.DependencyInfo(
                                                    sync=True,
                                                    no_sync=False))
                                    proj_pass(b, sh, rh, CW,
                                              b * 2 * CW + sh * CW,
                                              aux, outs)
    nc.compile()
    return nc


def _get_program(n_iters=1, phases=3, bench=False):
    key = (n_iters, phases, bench)
    if key not in _CACHE:
        _CACHE[key] = _build(n_iters, phases, bench)
    return _CACHE[key]


def _in_maps(x, wq, bq, wk, bk, wv, bv, wo, bo):
    x = np.asarray(x, np.float32)
    xT = np.ascontiguousarray(x.reshape(R, D).T.astype(np.float16))
    wo32 = np.asarray(wo, np.float32)
    # device layout [P, KC*D]: woT_sw[p, kc*D + c] = wo.T[kc*128 + p, c]
    woT = np.ascontiguousarray(
        wo32.T.astype(np.float16).reshape(KC, P, D).transpose(1, 0, 2)
        .reshape(P, KC * D))

    def _sw(w, sl):
        # device layout [P, D]: t[p, kc*128 + c] = w[sl][:, :].T[kc*128+p, c]
        wt = np.asarray(w, np.float32)[sl, :].T.astype(np.float16)
        return np.ascontiguousarray(
            wt.reshape(KC, P, DL).transpose(1, 0, 2).reshape(P, D))
    # bv folds into the output-projection bias: out = attn0 @ wo.T + (wo@bv+bo)
    bo_eff = np.asarray(bo, np.float32) + wo32 @ np.asarray(bv, np.float32)
    bo_t = np.ascontiguousarray(bo_eff.reshape(KC, P).T)
    maps = []
    for i in range(NCORES):
        sl = slice(i * DL, (i + 1) * DL)
        maps.append({
            "xT": xT,
            "wqT": _sw(wq, sl),
            "wkT": _sw(wk, sl),
            "wvT": _sw(wv, sl),
            "woT": woT,
            "bqk": np.ascontiguousarray(np.stack(
                [np.asarray(bq, np.float32)[sl],
                 np.asarray(bk, np.float32)[sl]], axis=1)),
            "bo_t": bo_t,
        })
    return maps


def kernel(x, wq, bq, wk, bk, wv, bv, wo, bo, **_):
    nc = _get_program()
    res = run_bass_kernel_spmd(nc, _in_maps(x, wq, bq, wk, bk, wv, bv, wo, bo),
                               list(range(NCORES)))
    # core j holds output columns [b*2048 + sh*1024 + j*128, +128) of out.T
    # at local columns b*256 + sh*128
    outT = np.empty((D, R), np.float32)
    for j in range(NCORES):
        o = res.results[j]["out"]
        for b in range(B):
            for sh in range(2):
                outT[:, b * S + sh * SH + j * CW:
                     b * S + sh * SH + (j + 1) * CW] = \
                    o[:, b * 256 + sh * CW:b * 256 + (sh + 1) * CW]
    return np.ascontiguousarray(outT.T).reshape(B, S, D)
